# revision 1
# baseline (speedup 1.0000x reference)
"""ColorUnpool (gather + segment-max + relu) as an 8-core Trainium2 Bass kernel.

Reference semantics:
    out = zeros([200000, 256]);  out[center_idx] = feat            # centers
    seg = segment_max(feat[edge_src], edge_dst)                    # edges
    out[r] = max(seg[r], 0) for rows r with >= 1 incoming edge

edge_dst only hits rows [50000, 200000) and center_idx only [0, 50000), so
the two regions are disjoint.  The center region is a pure host-side copy of
the input (no compute); the device computes the edge region only.

Device strategy (per core, rows split 8 ways -> 18750 dst rows/core):
  * Rows are degree-sorted (desc) and packed into 147 tiles of 128 rows.
    Tiles are dealt round-robin into NBLOCKS independent chains.
  * feat is converted to bf16 on the host (rel err ~4e-3 << 2e-2 gate) and
    gathered row-wise (512 B descriptors).  One *giant* indirect DMA per
    (block, round): round j gathers the j-th edge of every still-active row
    in the block, with SDMA inline CCE `max` accumulating directly into an
    SBUF accumulator (round 0 uses bypass to initialize).  This keeps the
    SWDGE descriptor-generation cost at ~40 instructions instead of ~550
    (994 ns fixed each + 0.34 ns/descriptor), which was the baseline's
    bottleneck (GpSimd busy 678 us of 915 us).
  * Rows with fewer edges than the round count gather a host-appended zero
    row: max(x, 0) is a no-op there (relu comes at the end anyway).
  * Epilogue: DVE relu chunks + dense contiguous SBUF->DRAM writes (no
    indirect scatter).  The host un-permutes rows and upcasts to f32.
"""

import sys
import types

import numpy as np
import ml_dtypes

sys.path.insert(0, "/opt/trn_rl_repo")

N_NODES = 200000
N_CENTERS = 50000
FEAT = 256
NCORES = 8
P = 128

R_EDGE = N_NODES - N_CENTERS          # 150000 edge-target rows
RC = R_EDGE // NCORES                 # 18750 edge rows per core
TILES = (RC + P - 1) // P             # 147 tiles of 128 rows
NPOS = TILES * P                      # 18816 padded row slots
ZROW = N_CENTERS                      # zero row appended to feat
NBLOCKS = 2
BSIZES = [len(range(b, TILES, NBLOCKS)) for b in range(NBLOCKS)]  # [74, 73]
BCOL0 = [sum(BSIZES[:b]) for b in range(NBLOCKS)]                 # acc col base


def _install_profile_hook():
    """Provide antenv.axon_hooks (missing on this image) so that
    run_bass_kernel_spmd(trace=True) can profile via the axon .so."""
    try:
        import antenv
        if "antenv.axon_hooks" in sys.modules:
            return
        from trn_agent_boot.trn_boot import _ntff_profile_via_ctypes
        mod = types.ModuleType("antenv.axon_hooks")
        hook = _ntff_profile_via_ctypes("/opt/axon/libaxon_pjrt.so")
        mod.get_axon_ntff_profile_hook = lambda: hook
        mod.set_axon_ntff_profile_hook = lambda h: None
        sys.modules["antenv.axon_hooks"] = mod
        antenv.axon_hooks = mod
    except Exception:
        pass


def _build_plan(edge_src, edge_dst):
    """Host preprocessing.

    Returns (instrs, C, in_maps_idx, orders) where
      instrs      = [(block, round, col_base, T)]  shared by all cores
      C           = total offset columns
      in_maps_idx = per-core offs arrays [P, C] int32 (feat row per slot)
      orders      = per-core position->local-row permutation [RC]
    """
    edge_src = np.asarray(edge_src, np.int64)
    edge_dst = np.asarray(edge_dst, np.int64)
    local_dst = edge_dst - N_CENTERS
    assert local_dst.min() >= 0 and local_dst.max() < R_EDGE
    core_of = local_dst // RC

    percore = []
    for c in range(NCORES):
        m = core_of == c
        ld = (local_dst[m] % RC).astype(np.int64)
        ss = edge_src[m].astype(np.int32)
        deg = np.bincount(ld, minlength=RC)
        order = np.argsort(-deg, kind="stable")          # rows desc by degree
        eo = np.argsort(ld, kind="stable")
        ss_sorted = ss[eo]                               # CSR values
        starts = np.concatenate([[0], np.cumsum(deg)[:-1]])
        deg_sorted = deg[order]
        # per-tile max degree (first row of each tile, desc order)
        d_tile = deg_sorted[np.arange(TILES) * P]
        percore.append(dict(deg=deg, order=order, ss=ss_sorted,
                            starts=starts, d_tile=d_tile))

    # union round counts per (block, round)
    maxd = max(int(pc["d_tile"][0]) for pc in percore)
    T_union = np.zeros((NBLOCKS, maxd), np.int64)
    for pc in percore:
        for b in range(NBLOCKS):
            db = pc["d_tile"][b::NBLOCKS]                # block tiles, desc
            for j in range(maxd):
                T_union[b, j] = max(T_union[b, j], int((db > j).sum()))
    # round 0 initializes (bypass): must cover every tile, incl. degree-0 and
    # padding tiles, which gather the zero row -> out 0
    for b in range(NBLOCKS):
        T_union[b, 0] = BSIZES[b]

    instrs = []
    col = 0
    for j in range(maxd):
        for b in range(NBLOCKS):
            T = int(T_union[b, j])
            if T > 0:
                instrs.append((b, j, col, T))
                col += T
    C = col

    offs_list = []
    for pc in percore:
        order_padded = np.full(NPOS, -1, np.int64)
        order_padded[:RC] = pc["order"]
        offs = np.empty((P, C), np.int32)
        deg = pc["deg"]
        starts = pc["starts"]
        ss = pc["ss"]
        pp = np.arange(P)
        for b, j, base, T in instrs:
            k = np.arange(T)
            t_global = k * NBLOCKS + b                   # [T]
            q = t_global[None, :] * P + pp[:, None]      # [P, T]
            r = order_padded[q]                          # [P, T] local row or -1
            rs = np.where(r >= 0, r, 0)
            has = (r >= 0) & (deg[rs] > j)
            src = np.where(has, ss[np.minimum(starts[rs] + j, len(ss) - 1)], ZROW)
            offs[:, base:base + T] = src.astype(np.int32)
        offs_list.append(offs)
    orders = [pc["order"] for pc in percore]
    return instrs, C, offs_list, orders


def _build_bass(instrs, C, n_epi_chunks=3, g_bufs=6, g_cap=37):
    import concourse.bass as bass
    import concourse.bacc as bacc
    import concourse.mybir as mybir
    import concourse.tile as tile

    nc = bacc.Bacc("TRN2", target_bir_lowering=False, debug=False,
                   num_devices=NCORES)
    t_feat = nc.dram_tensor("feat_aug", [N_CENTERS + 1, FEAT],
                            mybir.dt.bfloat16, kind="ExternalInput")
    t_offs = nc.dram_tensor("offs", [P, C], mybir.dt.int32,
                            kind="ExternalInput")
    t_oe = nc.dram_tensor("out_edge", [P, TILES * FEAT], mybir.dt.bfloat16,
                          kind="ExternalOutput")

    mx = mybir.AluOpType.max
    tmax = min(g_cap,
               max((T for _, j, _, T in instrs if j > 0), default=0))
    with tile.TileContext(nc) as tc:
        with tc.tile_pool(name="offp", bufs=1) as offp, \
             tc.tile_pool(name="accp", bufs=1) as accp, \
             tc.tile_pool(name="gp", bufs=g_bufs) as gp:
            offs = offp.tile([P, C], mybir.dt.int32)
            nc.sync.dma_start(out=offs[:], in_=t_offs[:])
            acc = accp.tile([P, TILES * FEAT], mybir.dt.bfloat16)

            # HW indirect DMA supports ONE offset per partition per
            # instruction (multi-index offset APs silently gather contiguous
            # runs instead) -> emit [128,1]-offset gathers, one per tile, but
            # keep the wide per-round DVE max and the dense epilogue.
            for b, j, base, T in instrs:
                c0 = BCOL0[b]
                if j == 0:
                    # round 0 initializes the whole block in place
                    for k in range(T):
                        nc.gpsimd.indirect_dma_start(
                            out=acc[:, (c0 + k) * FEAT:(c0 + k + 1) * FEAT],
                            out_offset=None,
                            in_=t_feat[:],
                            in_offset=bass.IndirectOffsetOnAxis(
                                ap=offs[:, base + k:base + k + 1], axis=0),
                        )
                else:
                    # chunk into <=tmax-tile groups, each with its own g
                    # buffer (deeper rotation -> fewer GpSimd reuse stalls)
                    for s in range(0, T, tmax):
                        W = min(tmax, T - s)
                        g = gp.tile([P, tmax * FEAT], mybir.dt.bfloat16,
                                    tag="g")
                        for k in range(s, s + W):
                            nc.gpsimd.indirect_dma_start(
                                out=g[:, (k - s) * FEAT:(k - s + 1) * FEAT],
                                out_offset=None,
                                in_=t_feat[:],
                                in_offset=bass.IndirectOffsetOnAxis(
                                    ap=offs[:, base + k:base + k + 1],
                                    axis=0),
                            )
                        nc.vector.tensor_tensor(
                            out=acc[:, (c0 + s) * FEAT:(c0 + s + W) * FEAT],
                            in0=acc[:, (c0 + s) * FEAT:(c0 + s + W) * FEAT],
                            in1=g[:, :W * FEAT], op=mx)

            # epilogue: relu + dense write, per block, chunked; high columns
            # (low-degree tiles) finish their rounds first -> emit those first
            for b in range(NBLOCKS):
                c0, B = BCOL0[b], BSIZES[b]
                bounds = np.linspace(0, B, n_epi_chunks + 1).astype(int)
                for ci in range(n_epi_chunks - 1, -1, -1):
                    lo = (c0 + bounds[ci]) * FEAT
                    hi = (c0 + bounds[ci + 1]) * FEAT
                    if hi <= lo:
                        continue
                    nc.vector.tensor_scalar_max(acc[:, lo:hi], acc[:, lo:hi],
                                                0.0)
                    nc.sync.dma_start(out=t_oe[:, lo:hi], in_=acc[:, lo:hi])
    nc.compile()
    return nc


def _unshard(results, orders, feat):
    out = np.empty((N_NODES, FEAT), np.float32)
    out[:N_CENTERS] = feat                               # centers: exact copy
    # acc col -> global tile: cols [BCOL0[b], BCOL0[b]+BSIZES[b]) hold tiles
    # b, b+NBLOCKS, ...
    col_to_tile = np.empty(TILES, np.int64)
    for b in range(NBLOCKS):
        col_to_tile[BCOL0[b]:BCOL0[b] + BSIZES[b]] = \
            np.arange(BSIZES[b]) * NBLOCKS + b
    tile_to_col = np.argsort(col_to_tile)                # global tile -> col
    for c in range(NCORES):
        oe = np.asarray(results[c]["out_edge"])          # [P, TILES*FEAT] bf16
        vals = oe.reshape(P, TILES, FEAT)[:, tile_to_col, :]   # [p, t, f]
        vals = vals.transpose(1, 0, 2).reshape(NPOS, FEAT)     # position-major
        rows = N_CENTERS + c * RC + orders[c]            # position q -> out row
        out[rows] = vals[:RC].astype(np.float32)
    return out


def kernel(feat, center_idx, edge_src, edge_dst, n_nodes, _trace=False):
    assert int(n_nodes) == N_NODES
    feat = np.ascontiguousarray(np.asarray(feat, np.float32))
    center_idx = np.asarray(center_idx, np.int64)

    # centers: out[center_idx] = feat, handled fully on the host (pure copy)
    feat_centers = np.zeros((N_CENTERS, FEAT), np.float32)
    feat_centers[center_idx] = feat

    instrs, C, offs_list, orders = _build_plan(edge_src, edge_dst)

    feat_aug = np.vstack([feat, np.zeros((1, FEAT), np.float32)])
    feat_aug = feat_aug.astype(ml_dtypes.bfloat16)

    nc = _build_bass(instrs, C)

    if _trace:
        _install_profile_hook()
    import concourse.bass_utils as bass_utils
    bass_utils.upload_artifacts = lambda tmpdir: f"file://{tmpdir}"
    from concourse.bass_utils import run_bass_kernel_spmd

    in_maps = [{"feat_aug": feat_aug, "offs": offs_list[c]}
               for c in range(NCORES)]
    kw = dict(trace=True) if _trace else {}
    res = run_bass_kernel_spmd(nc, in_maps, list(range(NCORES)), **kw)

    out = _unshard(res.results, orders, feat_centers)
    if _trace:
        return out, res
    return out



# revision 5
# speedup vs baseline: 1.3904x; 1.3904x over previous
"""ColorUnpool (gather + segment-max + relu) as an 8-core Trainium2 Bass kernel.

Reference semantics:
    out = zeros([200000, 256]);  out[center_idx] = feat            # centers
    seg = segment_max(feat[edge_src], edge_dst)                    # edges
    out[r] = max(seg[r], 0) for rows r with >= 1 incoming edge

edge_dst only hits rows [50000, 200000) and center_idx only [0, 50000), so
the two regions are disjoint.  The center region is a pure host-side copy of
the input (no compute); the device computes the edge region only.

Device strategy (per core, dst rows split 8 ways -> 18750 rows/core):
  * Rows are degree-sorted (desc) and packed into 147 tiles of 128 rows.
    Column layout is round-major: round 0 holds one column per tile (edge 0
    of every row, ZID pad for deg-0 rows); round j>=1 holds a column for
    each tile whose max degree exceeds j (a prefix, since tiles are sorted).
  * The feat table is compacted per core to its ~31.6k distinct src rows
    (< 32768), so gather indices fit in int16 and the whole gather runs as
    a handful of giant `dma_gather` instructions (994ns + 0.34ns/row SWDGE
    cost) instead of one 128-row indirect DMA per column (994ns each),
    which was the baseline's bottleneck (~410us serialized on Pool).
  * Round 0 gathers straight into the accumulator; rounds j>=1 gather into
    rotating SBUF chunks and fold in with fused DVE ops
    acc = max(max(acc, 0), g)  (scalar_tensor_tensor), which also bakes in
    the final relu.  Tiles only touched by round 0 get an Activation-engine
    relu instead.  Finished tile ranges are written back to DRAM as soon as
    their last round completes, overlapping the output DMA with the
    remaining gathers.
  * feat is bf16 on device (rel err ~4e-3 << 2e-2 gate); the host
    un-permutes rows and upcasts to f32.
"""

import sys
import types

import numpy as np
import ml_dtypes

sys.path.insert(0, "/opt/trn_rl_repo")

N_NODES = 200000
N_CENTERS = 50000
FEAT = 256
NCORES = 8
P = 128

R_EDGE = N_NODES - N_CENTERS          # 150000 edge-target rows
RC = R_EDGE // NCORES                 # 18750 edge rows per core
TILES = (RC + P - 1) // P             # 147 tiles of 128 rows
NPOS = TILES * P                      # 18816 padded row slots
TBL = 32768                           # per-core compact feat table rows
ZID = TBL - 1                         # zero row id (table is zero-padded)
G = 8                                 # gather chunk width (cols); HW caps a
                                      # single dma_gather at 1024 indices
WMIN = 8                              # min writeback width (tiles)


def _install_profile_hook():
    """Provide antenv.axon_hooks (missing on this image) so that
    run_bass_kernel_spmd(trace=True) can profile via the axon .so."""
    try:
        import antenv
        if "antenv.axon_hooks" in sys.modules:
            return
        from trn_agent_boot.trn_boot import _ntff_profile_via_ctypes
        mod = types.ModuleType("antenv.axon_hooks")
        hook = _ntff_profile_via_ctypes("/opt/axon/libaxon_pjrt.so")
        mod.get_axon_ntff_profile_hook = lambda: hook
        mod.set_axon_ntff_profile_hook = lambda h: None
        sys.modules["antenv.axon_hooks"] = mod
        antenv.axon_hooks = mod
    except Exception:
        pass


def _build_plan(edge_src, edge_dst, feat):
    """Host preprocessing.

    Returns (T, bases, C, tables, idx_planes, orders):
      T          = per-round union active-tile counts, T[0] == TILES
      bases      = column base per round
      C          = total columns
      tables     = per-core compact bf16 feat tables [TBL, FEAT]
      idx_planes = per-core int16 idx planes [P, C*8] (x8 Q7 replication)
      orders     = per-core position->local-row permutation [RC]
    """
    edge_src = np.asarray(edge_src, np.int64)
    edge_dst = np.asarray(edge_dst, np.int64)
    local_dst = edge_dst - N_CENTERS
    assert local_dst.min() >= 0 and local_dst.max() < R_EDGE
    core_of = local_dst // RC

    percore = []
    for c in range(NCORES):
        m = core_of == c
        ld = (local_dst[m] % RC).astype(np.int64)
        ss = edge_src[m].astype(np.int64)
        deg = np.bincount(ld, minlength=RC)
        order = np.argsort(-deg, kind="stable")          # rows desc by degree
        eo = np.argsort(ld, kind="stable")
        ss_sorted = ss[eo]                               # CSR values
        starts = np.concatenate([[0], np.cumsum(deg)[:-1]])
        uniq, inv = np.unique(ss_sorted, return_inverse=True)
        assert len(uniq) < TBL, f"core {c}: {len(uniq)} distinct srcs > int16"
        ssc = inv.astype(np.int64)                       # compact CSR values
        deg_sorted = deg[order]
        d_tile = deg_sorted[np.arange(TILES) * P]        # per-tile max degree
        percore.append(dict(deg=deg, order=order, ssc=ssc, starts=starts,
                            d_tile=d_tile, uniq=uniq))

    maxd = max(max(int(pc["d_tile"][0]), 1) for pc in percore)
    T = [TILES]                                          # round 0: all tiles
    for j in range(1, maxd):
        T.append(max(int((pc["d_tile"] > j).sum()) for pc in percore))
    bases = np.concatenate([[0], np.cumsum(T)[:-1]]).astype(int)
    C = int(np.sum(T))

    tables, idx_planes, orders = [], [], []
    for pc in percore:
        order_padded = np.full(NPOS, -1, np.int64)
        order_padded[:RC] = pc["order"]
        deg, starts, ssc = pc["deg"], pc["starts"], pc["ssc"]
        vals = np.full(C * P, ZID, np.int64)
        for j in range(maxd):
            qpos = np.arange(T[j] * P)
            r = order_padded[qpos]
            rs = np.where(r >= 0, r, 0)
            has = (r >= 0) & (deg[rs] > j)
            v = np.where(has, ssc[np.minimum(starts[rs] + j, len(ssc) - 1)],
                         ZID)
            vals[bases[j] * P: bases[j] * P + T[j] * P] = v
        # idx position g lives at [g%16, g//16], replicated x8 for Q7 cores
        plane16 = vals.astype(np.int16).reshape(C * 8, 16).T
        idx_planes.append(np.ascontiguousarray(np.tile(plane16, (8, 1))))
        tbl = np.zeros((TBL, FEAT), ml_dtypes.bfloat16)
        tbl[:len(pc["uniq"])] = feat[pc["uniq"]].astype(ml_dtypes.bfloat16)
        tables.append(tbl)
        orders.append(pc["order"])
    return T, bases, C, tables, idx_planes, orders


def _build_bass(T, bases, C):
    import concourse.bacc as bacc
    import concourse.mybir as mybir
    import concourse.tile as tile

    maxd = len(T)
    nc = bacc.Bacc("TRN2", target_bir_lowering=False, debug=False,
                   num_devices=NCORES)
    t_feat = nc.dram_tensor("feat_tbl", [TBL, FEAT], mybir.dt.bfloat16,
                            kind="ExternalInput")
    t_idx = nc.dram_tensor("idxs", [P, C * 8], mybir.dt.int16,
                           kind="ExternalInput")
    t_oe = nc.dram_tensor("out_edge", [P, TILES, FEAT], mybir.dt.bfloat16,
                          kind="ExternalOutput")

    mx = mybir.AluOpType.max
    relu = mybir.ActivationFunctionType.Relu

    # G-column chunks, split at the round-0 boundary (those go straight
    # into the accumulator)
    chunks = []
    s = 0
    while s < C:
        e = min(s + G, TILES if s < TILES else C)
        chunks.append((s, e))
        s = e

    # last chunk index per round (where its final column lands)
    end_chunk = {}
    for j in range(maxd):
        last_col = bases[j] + T[j] - 1
        for k, (cs, ce) in enumerate(chunks):
            if cs <= last_col < ce:
                end_chunk[j] = k

    with tile.TileContext(nc) as tc:
        with tc.tile_pool(name="idxp", bufs=1) as idxp, \
             tc.tile_pool(name="accp", bufs=1) as accp, \
             tc.tile_pool(name="gp", bufs=4) as gp:
            idx = idxp.tile([P, C * 8], mybir.dt.int16)
            nc.sync.dma_start(out=idx[:], in_=t_idx[:])
            acc = accp.tile([P, TILES, FEAT], mybir.dt.bfloat16)

            pend_lo = TILES  # writeback merge: pending final range [lo, hi)
            pend_hi = TILES
            for k, (cs, ce) in enumerate(chunks):
                w = ce - cs
                if ce <= TILES:                          # round 0: direct
                    gout = acc[:, cs:ce, :]
                else:
                    g = gp.tile([P, G, FEAT], mybir.dt.bfloat16, tag="g")
                    gout = g[:, :w, :]
                nc.gpsimd.dma_gather(gout, t_feat[:], idx[:, cs * 8:ce * 8],
                                     w * P, w * P, FEAT)
                # fused max+relu pieces for rounds j>=1 covered by the chunk
                for j in range(1, maxd):
                    a = max(cs, int(bases[j]))
                    b = min(ce, int(bases[j]) + T[j])
                    if a < b:
                        t0 = a - int(bases[j])
                        L = b - a
                        nc.vector.scalar_tensor_tensor(
                            out=acc[:, t0:t0 + L, :],
                            in0=acc[:, t0:t0 + L, :], scalar=0.0,
                            in1=g[:, a - cs:b - cs, :], op0=mx, op1=mx)
                # writebacks for rounds that completed with this chunk
                for j in range(maxd):
                    if end_chunk.get(j) != k:
                        continue
                    lo = T[j + 1] if j + 1 < maxd else 0
                    if j == 0 and lo < TILES:
                        # round-0-only tiles: relu never fused -> Act engine
                        nc.scalar.activation(acc[:, lo:TILES, :],
                                             acc[:, lo:TILES, :], relu)
                    pend_lo = min(pend_lo, lo)
                    final = k == len(chunks) - 1
                    if pend_hi - pend_lo >= WMIN or (final and
                                                     pend_hi > pend_lo):
                        nc.sync.dma_start(out=t_oe[:, pend_lo:pend_hi, :],
                                          in_=acc[:, pend_lo:pend_hi, :])
                        pend_hi = pend_lo
                if k == len(chunks) - 1 and pend_hi > pend_lo:
                    nc.sync.dma_start(out=t_oe[:, pend_lo:pend_hi, :],
                                      in_=acc[:, pend_lo:pend_hi, :])
    nc.compile()
    return nc


def _unshard(results, orders, feat_centers):
    out = np.empty((N_NODES, FEAT), np.float32)
    out[:N_CENTERS] = feat_centers                       # centers: exact copy
    for c in range(NCORES):
        oe = np.asarray(results[c]["out_edge"])          # [P, TILES, FEAT]
        vals = oe.transpose(1, 0, 2).reshape(NPOS, FEAT)  # position-major
        rows = N_CENTERS + c * RC + orders[c]            # position q -> row
        out[rows] = vals[:RC].astype(np.float32)
    return out


def kernel(feat, center_idx, edge_src, edge_dst, n_nodes, _trace=False):
    assert int(n_nodes) == N_NODES
    feat = np.ascontiguousarray(np.asarray(feat, np.float32))
    center_idx = np.asarray(center_idx, np.int64)

    # centers: out[center_idx] = feat, handled fully on the host (pure copy)
    feat_centers = np.zeros((N_CENTERS, FEAT), np.float32)
    feat_centers[center_idx] = feat

    T, bases, C, tables, idx_planes, orders = _build_plan(edge_src, edge_dst,
                                                          feat)
    nc = _build_bass(T, bases, C)

    if _trace:
        _install_profile_hook()
    import concourse.bass_utils as bass_utils
    bass_utils.upload_artifacts = lambda tmpdir: f"file://{tmpdir}"
    from concourse.bass_utils import run_bass_kernel_spmd

    in_maps = [{"feat_tbl": tables[c], "idxs": idx_planes[c]}
               for c in range(NCORES)]
    kw = dict(trace=True) if _trace else {}
    res = run_bass_kernel_spmd(nc, in_maps, list(range(NCORES)), **kw)

    out = _unshard(res.results, orders, feat_centers)
    if _trace:
        return out, res
    return out


# revision 8
# speedup vs baseline: 3.5404x; 2.5463x over previous
"""ColorUnpool (gather + segment-max + relu) as an 8-core Trainium2 Bass kernel.

Reference semantics:
    out = zeros([200000, 256]);  out[center_idx] = feat            # centers
    seg = segment_max(feat[edge_src], edge_dst)                    # edges
    out[r] = max(seg[r], 0) for rows r with >= 1 incoming edge

edge_dst only hits rows [50000, 200000) and center_idx only [0, 50000), so
the two regions are disjoint.  The center region is a pure host-side copy of
the input (no compute); the device computes the edge region only.

Device strategy (per core, dst rows split 8 ways -> 18750 rows/core):
  * Rows are degree-sorted (desc) and packed into 147 tiles of 128 rows.
    Column layout is round-major: round 0 holds one column per tile (edge 0
    of every row, ZID pad for deg-0 rows); round j>=1 holds a column for
    each tile whose max degree exceeds j (a prefix, since tiles are sorted).
  * The feat table is compacted per core to its ~31.6k distinct src rows
    (< 32768), so gather indices fit in int16 and the whole gather runs as
    a handful of giant `dma_gather` instructions (994ns + 0.34ns/row SWDGE
    cost) instead of one 128-row indirect DMA per column (994ns each),
    which was the baseline's bottleneck (~410us serialized on Pool).
  * Round 0 gathers straight into the accumulator; rounds j>=1 gather into
    rotating SBUF chunks and fold in with fused DVE ops
    acc = max(max(acc, 0), g)  (scalar_tensor_tensor), which also bakes in
    the final relu.  Tiles only touched by round 0 get an Activation-engine
    relu instead.  Finished tile ranges are written back to DRAM as soon as
    their last round completes, overlapping the output DMA with the
    remaining gathers.
  * feat is bf16 on device (rel err ~4e-3 << 2e-2 gate); the host
    un-permutes rows and upcasts to f32.
"""

import sys
import types

import numpy as np
import ml_dtypes

sys.path.insert(0, "/opt/trn_rl_repo")

N_NODES = 200000
N_CENTERS = 50000
FEAT = 256
NCORES = 8
P = 128

R_EDGE = N_NODES - N_CENTERS          # 150000 edge-target rows
RC = R_EDGE // NCORES                 # 18750 edge rows per core
TILES = (RC + P - 1) // P             # 147 tiles of 128 rows
NPOS = TILES * P                      # 18816 padded row slots
TBL = 32768                           # per-core compact feat table rows
ZID = TBL - 1                         # zero row id (table is zero-padded)
G = 8                                 # gather chunk width (cols); HW caps a
                                      # single dma_gather at 1024 indices
WMIN = 8                              # min writeback width (tiles)


def _install_profile_hook():
    """Provide antenv.axon_hooks (missing on this image) so that
    run_bass_kernel_spmd(trace=True) can profile via the axon .so."""
    try:
        import antenv
        if "antenv.axon_hooks" in sys.modules:
            return
        from trn_agent_boot.trn_boot import _ntff_profile_via_ctypes
        mod = types.ModuleType("antenv.axon_hooks")
        hook = _ntff_profile_via_ctypes("/opt/axon/libaxon_pjrt.so")
        mod.get_axon_ntff_profile_hook = lambda: hook
        mod.set_axon_ntff_profile_hook = lambda h: None
        sys.modules["antenv.axon_hooks"] = mod
        antenv.axon_hooks = mod
    except Exception:
        pass


def _build_plan(edge_src, edge_dst, feat):
    """Host preprocessing.

    Returns (T, bases, C, tables, idx_planes, orders):
      T          = per-round union active-tile counts, T[0] == TILES
      bases      = column base per round
      C          = total columns
      tables     = per-core compact bf16 feat tables [TBL, FEAT]
      idx_planes = per-core int16 idx planes [P, C*8] (x8 Q7 replication)
      orders     = per-core position->local-row permutation [RC]
    """
    edge_src = np.asarray(edge_src, np.int64)
    edge_dst = np.asarray(edge_dst, np.int64)
    local_dst = edge_dst - N_CENTERS
    assert local_dst.min() >= 0 and local_dst.max() < R_EDGE
    core_of = local_dst // RC

    percore = []
    for c in range(NCORES):
        m = core_of == c
        ld = (local_dst[m] % RC).astype(np.int64)
        ss = edge_src[m].astype(np.int64)
        deg = np.bincount(ld, minlength=RC)
        order = np.argsort(-deg, kind="stable")          # rows desc by degree
        eo = np.argsort(ld, kind="stable")
        ss_sorted = ss[eo]                               # CSR values
        starts = np.concatenate([[0], np.cumsum(deg)[:-1]])
        uniq, inv = np.unique(ss_sorted, return_inverse=True)
        assert len(uniq) < TBL, f"core {c}: {len(uniq)} distinct srcs > int16"
        ssc = inv.astype(np.int64)                       # compact CSR values
        deg_sorted = deg[order]
        d_tile = deg_sorted[np.arange(TILES) * P]        # per-tile max degree
        percore.append(dict(deg=deg, order=order, ssc=ssc, starts=starts,
                            d_tile=d_tile, uniq=uniq))

    maxd = max(max(int(pc["d_tile"][0]), 1) for pc in percore)
    T = [TILES]                                          # round 0: all tiles
    for j in range(1, maxd):
        T.append(max(int((pc["d_tile"] > j).sum()) for pc in percore))
    bases = np.concatenate([[0], np.cumsum(T)[:-1]]).astype(int)
    C = int(np.sum(T))

    tables, idx_planes, orders = [], [], []
    for pc in percore:
        order_padded = np.full(NPOS, -1, np.int64)
        order_padded[:RC] = pc["order"]
        deg, starts, ssc = pc["deg"], pc["starts"], pc["ssc"]
        vals = np.full(C * P, ZID, np.int64)
        for j in range(maxd):
            qpos = np.arange(T[j] * P)
            r = order_padded[qpos]
            rs = np.where(r >= 0, r, 0)
            has = (r >= 0) & (deg[rs] > j)
            v = np.where(has, ssc[np.minimum(starts[rs] + j, len(ssc) - 1)],
                         ZID)
            vals[bases[j] * P: bases[j] * P + T[j] * P] = v
        # idx position g lives at [g%16, g//16], replicated x8 for Q7 cores
        plane16 = vals.astype(np.int16).reshape(C * 8, 16).T
        idx_planes.append(np.ascontiguousarray(np.tile(plane16, (8, 1))))
        tbl = np.zeros((TBL, FEAT), ml_dtypes.bfloat16)
        tbl[:len(pc["uniq"])] = feat[pc["uniq"]].astype(ml_dtypes.bfloat16)
        tables.append(tbl)
        orders.append(pc["order"])
    return T, bases, C, tables, idx_planes, orders


def _build_bass(T, bases, C):
    import concourse.bacc as bacc
    import concourse.mybir as mybir
    import concourse.tile as tile

    maxd = len(T)
    nc = bacc.Bacc("TRN2", target_bir_lowering=False, debug=False,
                   num_devices=NCORES, num_swdge_queues=4)
    t_feat = nc.dram_tensor("feat_tbl", [TBL, FEAT], mybir.dt.bfloat16,
                            kind="ExternalInput")
    t_idx = nc.dram_tensor("idxs", [P, C * 8], mybir.dt.int16,
                           kind="ExternalInput")
    t_oe = nc.dram_tensor("out_edge", [P, TILES, FEAT], mybir.dt.bfloat16,
                          kind="ExternalOutput")

    mx = mybir.AluOpType.max
    relu = mybir.ActivationFunctionType.Relu

    # G-column chunks, split at the round-0 boundary (those go straight
    # into the accumulator)
    chunks = []
    s = 0
    while s < C:
        e = min(s + G, TILES if s < TILES else C)
        chunks.append((s, e))
        s = e

    # last chunk index per round (where its final column lands)
    end_chunk = {}
    for j in range(maxd):
        last_col = bases[j] + T[j] - 1
        for k, (cs, ce) in enumerate(chunks):
            if cs <= last_col < ce:
                end_chunk[j] = k

    with tile.TileContext(nc) as tc:
        with tc.tile_pool(name="idxp", bufs=1) as idxp, \
             tc.tile_pool(name="accp", bufs=1) as accp, \
             tc.tile_pool(name="gp", bufs=8) as gp:
            idx = idxp.tile([P, C * 8], mybir.dt.int16)
            nc.sync.dma_start(out=idx[:], in_=t_idx[:])
            acc = accp.tile([P, TILES, FEAT], mybir.dt.bfloat16)

            pend_lo = TILES  # writeback merge: pending final range [lo, hi)
            pend_hi = TILES
            for k, (cs, ce) in enumerate(chunks):
                w = ce - cs
                if ce <= TILES:                          # round 0: direct
                    gout = acc[:, cs:ce, :]
                else:
                    g = gp.tile([P, G, FEAT], mybir.dt.bfloat16, tag="g")
                    gout = g[:, :w, :]
                nc.gpsimd.dma_gather(gout, t_feat[:], idx[:, cs * 8:ce * 8],
                                     w * P, w * P, FEAT, queue_num=k % 4)
                # fused max+relu pieces for rounds j>=1 covered by the chunk
                for j in range(1, maxd):
                    a = max(cs, int(bases[j]))
                    b = min(ce, int(bases[j]) + T[j])
                    if a < b:
                        t0 = a - int(bases[j])
                        L = b - a
                        nc.vector.scalar_tensor_tensor(
                            out=acc[:, t0:t0 + L, :],
                            in0=acc[:, t0:t0 + L, :], scalar=0.0,
                            in1=g[:, a - cs:b - cs, :], op0=mx, op1=mx)
                # writebacks for rounds that completed with this chunk
                for j in range(maxd):
                    if end_chunk.get(j) != k:
                        continue
                    lo = T[j + 1] if j + 1 < maxd else 0
                    if j == 0 and lo < TILES:
                        # round-0-only tiles: relu never fused -> Act engine
                        nc.scalar.activation(acc[:, lo:TILES, :],
                                             acc[:, lo:TILES, :], relu)
                    pend_lo = min(pend_lo, lo)
                    final = k == len(chunks) - 1
                    if pend_hi - pend_lo >= WMIN or (final and
                                                     pend_hi > pend_lo):
                        nc.sync.dma_start(out=t_oe[:, pend_lo:pend_hi, :],
                                          in_=acc[:, pend_lo:pend_hi, :])
                        pend_hi = pend_lo
                if k == len(chunks) - 1 and pend_hi > pend_lo:
                    nc.sync.dma_start(out=t_oe[:, pend_lo:pend_hi, :],
                                      in_=acc[:, pend_lo:pend_hi, :])
    nc.compile()
    return nc


def _unshard(results, orders, feat_centers):
    out = np.empty((N_NODES, FEAT), np.float32)
    out[:N_CENTERS] = feat_centers                       # centers: exact copy
    for c in range(NCORES):
        oe = np.asarray(results[c]["out_edge"])          # [P, TILES, FEAT]
        vals = oe.transpose(1, 0, 2).reshape(NPOS, FEAT)  # position-major
        rows = N_CENTERS + c * RC + orders[c]            # position q -> row
        out[rows] = vals[:RC].astype(np.float32)
    return out


def kernel(feat, center_idx, edge_src, edge_dst, n_nodes, _trace=False):
    assert int(n_nodes) == N_NODES
    feat = np.ascontiguousarray(np.asarray(feat, np.float32))
    center_idx = np.asarray(center_idx, np.int64)

    # centers: out[center_idx] = feat, handled fully on the host (pure copy)
    feat_centers = np.zeros((N_CENTERS, FEAT), np.float32)
    feat_centers[center_idx] = feat

    T, bases, C, tables, idx_planes, orders = _build_plan(edge_src, edge_dst,
                                                          feat)
    nc = _build_bass(T, bases, C)

    if _trace:
        _install_profile_hook()
    import concourse.bass_utils as bass_utils
    bass_utils.upload_artifacts = lambda tmpdir: f"file://{tmpdir}"
    from concourse.bass_utils import run_bass_kernel_spmd

    in_maps = [{"feat_tbl": tables[c], "idxs": idx_planes[c]}
               for c in range(NCORES)]
    kw = dict(trace=True) if _trace else {}
    res = run_bass_kernel_spmd(nc, in_maps, list(range(NCORES)), **kw)

    out = _unshard(res.results, orders, feat_centers)
    if _trace:
        return out, res
    return out
